# revision 1
# baseline (speedup 1.0000x reference)
"""ExLlama q4 dequant + matmul, 8 trn2 cores — hybrid + pair-merged DVE ops.

Like kernel_d (hosted fp16 nibble planes for HJT jts + packed u16 containers
for the rest), but DVE tensor_tensor ops process TWO passes at once against a
resident doubled scale table [sexp|sexp], halving DVE op count; packed
extracts write into halves of a pair buffer.  For_i uses staggered_reset.
"""

import numpy as np

GROUP_SIZE = 128
IN_FEATURES = 8192
OUT_FEATURES = 28672
TOKENS = 32
N_CORES = 8
N_LOC = OUT_FEATURES // N_CORES          # 3584
NJT = IN_FEATURES // (GROUP_SIZE * 4)    # 16
G = IN_FEATURES // GROUP_SIZE            # 64
NPASS = NJT * 4
MMCH = 512

HJT = 8                                        # hosted jts (planes)
PJT = NJT - HJT
PACKED_JTS = sorted({round((i + 0.5) * NJT / PJT) % NJT
                     for i in range(PJT)}) if PJT else []
while len(PACKED_JTS) < PJT:
    PACKED_JTS.append(next(j for j in range(NJT) if j not in PACKED_JTS))
PACKED_JTS = sorted(PACKED_JTS[:PJT])
HOSTED_JTS = [j for j in range(NJT) if j not in PACKED_JTS]

PLANE_CH = 4                                   # hosted passes per plane DMA
PACK_CH = 2                                    # packed jts per container DMA
N_PLANE_DMA = max((HJT * 4) // PLANE_CH, 1)
N_PACK_DMA = (PJT + PACK_CH - 1) // PACK_CH if PJT else 1

# packed const layout (columns, fp16): [sexp2 | xt | z65 | r65]
C_SEXP = 0                                     # doubled: 2*N_LOC wide
C_XT = 2 * N_LOC
C_Z65 = C_XT + NPASS * TOKENS
C_R65 = C_Z65 + N_LOC
C_W = C_R65 + TOKENS

MASKS = (0x000F, 0x00F0, 0x0F00, 0xF000)
_PROGRAM_CACHE = {}


def _k_index_map():
    jt = np.arange(NJT)[:, None, None]
    p = np.arange(128)[None, :, None]
    c = np.arange(4)[None, None, :]
    return (p // 2) * GROUP_SIZE + (jt * 2 + (p % 2)) * 4 + c


def _containers(qw_slice):
    nloc = qw_slice.shape[1]
    qb = np.ascontiguousarray(qw_slice).view(np.uint8).reshape(1024, nloc, 4)
    qb_kp = np.ascontiguousarray(qb.transpose(0, 2, 1)).reshape(4096, nloc)
    jt = np.arange(NJT)[:, None]
    p = np.arange(128)[None, :]
    kp0 = (p // 2) * 64 + (jt * 2 + (p % 2)) * 2
    b2 = np.stack([qb_kp[kp0], qb_kp[kp0 + 1]], axis=-1)
    return np.ascontiguousarray(b2).view(np.uint16)[..., 0]


def _prep_weights(qw_slice):
    nloc = qw_slice.shape[1]
    wq = _containers(qw_slice)
    hp = np.empty((max(HJT * 4, PLANE_CH), 128, nloc), dtype=np.float16)
    for i, jt in enumerate(HOSTED_JTS):
        for c in range(4):
            hp[i * 4 + c] = ((wq[jt] >> (4 * c)) & 15).astype(np.float16)
    hp = hp[:N_PLANE_DMA * PLANE_CH]
    hp = hp.reshape(N_PLANE_DMA, PLANE_CH, 128, nloc).transpose(0, 2, 1, 3)
    planes = np.ascontiguousarray(hp).reshape(N_PLANE_DMA, 128,
                                              PLANE_CH * nloc)
    pk = np.zeros((N_PACK_DMA * PACK_CH, 128, nloc), dtype=np.uint16)
    for i, jt in enumerate(PACKED_JTS):
        pk[i] = wq[jt]
    pk = pk.reshape(N_PACK_DMA, PACK_CH, 128, nloc).transpose(0, 2, 1, 3)
    packs = np.ascontiguousarray(pk).reshape(N_PACK_DMA, 128, PACK_CH * nloc)
    return planes, packs


def _prep_const(x, qz_slice, s_slice, b_slice):
    nloc = s_slice.shape[1]
    cst = np.zeros((128, C_W), dtype=np.float16)
    sexp = np.repeat(s_slice.astype(np.float16), 2, axis=0)
    cst[:, C_SEXP:C_SEXP + nloc] = sexp
    cst[:, C_SEXP + nloc:C_SEXP + 2 * nloc] = sexp
    kmap = _k_index_map()
    xf = x.astype(np.float32)
    packed = set(PACKED_JTS)
    for jt in range(NJT):
        for c in range(4):
            col = C_XT + (jt * 4 + c) * TOKENS
            scale = (2.0 ** (-4 * c)) if jt in packed else 1.0
            cst[:, col:col + TOKENS] = (xf[:, kmap[jt, :, c]].T * scale
                                        ).astype(np.float16)
    shifts = (np.arange(8, dtype=np.uint32) * 4)[None, None, :]
    z = ((qz_slice.astype(np.uint32)[:, :, None] >> shifts) & 15)
    z = z.reshape(G, nloc).astype(np.float32)
    cst[:G, C_Z65:C_Z65 + nloc] = ((z + 1.0) * s_slice.astype(np.float32)
                                   ).astype(np.float16)
    cst[G, C_Z65:C_Z65 + nloc] = b_slice
    A = x.astype(np.float32).reshape(TOKENS, G, GROUP_SIZE).sum(axis=2)
    cst[:G, C_R65:C_R65 + TOKENS] = (-A.T).astype(np.float16)
    cst[G, C_R65:C_R65 + TOKENS] = 1.0
    return cst


def _build_program(nloc, loop_r=1):
    import concourse.bacc as bacc
    import concourse.mybir as mybir
    import concourse.tile as tile
    from concourse.alu_op_type import AluOpType

    dt = mybir.dt
    nch = nloc // MMCH

    nc = bacc.Bacc("TRN2", target_bir_lowering=False, debug=False,
                   num_devices=N_CORES)

    wp_d = nc.dram_tensor("wp", [N_PLANE_DMA, 128, PLANE_CH * nloc],
                          dt.float16, kind="ExternalInput")
    pk_d = nc.dram_tensor("pk", [N_PACK_DMA, 128, PACK_CH * nloc],
                          dt.uint16, kind="ExternalInput")
    cst_d = nc.dram_tensor("cst", [128, C_W], dt.float16,
                           kind="ExternalInput")
    out_d = nc.dram_tensor("out", [TOKENS, nloc], dt.float16,
                           kind="ExternalOutput")

    with tile.TileContext(nc) as tc:
        with (
            tc.tile_pool(name="const", bufs=1) as const_pool,
            tc.tile_pool(name="wp", bufs=2) as wp_pool,
            tc.tile_pool(name="pk", bufs=2) as pk_pool,
            tc.tile_pool(name="ext", bufs=2) as ext_pool,
            tc.tile_pool(name="sw", bufs=3) as sw_pool,
            tc.tile_pool(name="psum", bufs=1, space="PSUM") as psum_pool,
        ):
            def emit_body():
                cst = const_pool.tile([128, C_W], dt.float16, tag="cst")
                nc.sync.dma_start(cst[:, C_SEXP:C_SEXP + 2 * nloc],
                                  cst_d[:, C_SEXP:C_SEXP + 2 * nloc])
                nc.sync.dma_start(cst[:, C_XT:C_W], cst_d[:, C_XT:C_W])
                sexp2 = cst[:, C_SEXP:C_SEXP + 2 * nloc]
                psum = psum_pool.tile([TOKENS, nch * MMCH], dt.float32,
                                      tag="acc")

                state = {"first": True}

                def matmuls(ip, sw, off):
                    xcol = C_XT + ip * TOKENS
                    for ci in range(nch):
                        nc.tensor.matmul(
                            psum[:, ci * MMCH:(ci + 1) * MMCH],
                            cst[:, xcol:xcol + TOKENS],
                            sw[:, off + ci * MMCH:off + (ci + 1) * MMCH],
                            start=state["first"],
                            stop=False)
                    state["first"] = False

                hosted_seq = [(jt, c) for jt in HOSTED_JTS for c in range(4)]
                hp_i = pk_i = 0
                plane_tiles = []
                pack_tiles = []
                hpos = 0
                ppos = 0
                total = len(hosted_seq) + 4 * len(PACKED_JTS)
                emitted = 0
                while emitted < total:
                    want_hosted = (hpos * total <=
                                   (hpos + 4 * ppos) * max(len(hosted_seq), 1))
                    if want_hosted and hpos < len(hosted_seq):
                        if hpos % PLANE_CH == 0:
                            t = wp_pool.tile([128, PLANE_CH * nloc],
                                             dt.float16)
                            nc.sync.dma_start(t[:], wp_d[hp_i, :, :])
                            plane_tiles.append(t)
                            hp_i += 1
                        t = plane_tiles[-1]
                        j = hpos % PLANE_CH          # 0 or 2 (pair-aligned)
                        sw = sw_pool.tile([128, 2 * nloc], dt.float16)
                        nc.vector.tensor_tensor(
                            sw[:], t[:, j * nloc:(j + 2) * nloc],
                            sexp2, AluOpType.mult)
                        for u in range(2):
                            jt, c = hosted_seq[hpos + u]
                            matmuls(jt * 4 + c, sw, u * nloc)
                        hpos += 2
                        emitted += 2
                    elif ppos < len(PACKED_JTS):
                        if ppos % PACK_CH == 0:
                            t = pk_pool.tile([128, PACK_CH * nloc], dt.uint16)
                            nc.sync.dma_start(t[:], pk_d[pk_i, :, :])
                            pack_tiles.append(t)
                            pk_i += 1
                        t = pack_tiles[-1]
                        j = ppos % PACK_CH
                        jt = PACKED_JTS[ppos]
                        wsl = t[:, j * nloc:(j + 1) * nloc]
                        for cp in range(2):          # c pairs (0,1), (2,3)
                            ext = ext_pool.tile([128, 2 * nloc], dt.uint16)
                            nc.vector.tensor_scalar(
                                ext[:, 0:nloc], wsl, MASKS[2 * cp], None,
                                AluOpType.bitwise_and)
                            nc.vector.tensor_scalar(
                                ext[:, nloc:2 * nloc], wsl,
                                MASKS[2 * cp + 1], None,
                                AluOpType.bitwise_and)
                            sw = sw_pool.tile([128, 2 * nloc], dt.float16)
                            nc.vector.tensor_tensor(
                                sw[:], ext[:], sexp2, AluOpType.mult)
                            for u in range(2):
                                matmuls(jt * 4 + 2 * cp + u, sw, u * nloc)
                            emitted += 2
                        ppos += 1
                    else:
                        continue

                for ci in range(nch):
                    nc.tensor.matmul(
                        psum[:, ci * MMCH:(ci + 1) * MMCH],
                        cst[0:G + 1, C_R65:C_R65 + TOKENS],
                        cst[0:G + 1, C_Z65 + ci * MMCH:C_Z65 + (ci + 1) * MMCH],
                        start=False,
                        stop=True)

                stg = const_pool.tile([TOKENS, nch * MMCH], dt.float16,
                                      tag="stg")
                nc.scalar.copy(stg[:], psum[:])
                nc.sync.dma_start(out_d[:], stg[:])

            if loop_r == 1:
                emit_body()
            else:
                with tc.For_i(0, loop_r, 1, staggered_reset=True):
                    emit_body()

    nc.compile()
    return nc


def _get_program(nloc=N_LOC):
    if nloc not in _PROGRAM_CACHE:
        _PROGRAM_CACHE[nloc] = _build_program(nloc)
    return _PROGRAM_CACHE[nloc]


def make_in_maps(x, qweight, qzeros, scales, bias, nloc=N_LOC, n_cores=N_CORES):
    x = np.asarray(x)
    qweight = np.asarray(qweight)
    qzeros = np.asarray(qzeros)
    scales = np.asarray(scales)
    bias = np.asarray(bias)

    in_maps = []
    for core in range(n_cores):
        n0, n1 = core * nloc, (core + 1) * nloc
        s_slice = np.ascontiguousarray(scales[:, n0:n1]).astype(np.float16)
        qz_slice = np.ascontiguousarray(qzeros[:, n0 // 8:n1 // 8]).view(
            np.uint32)
        b_slice = np.ascontiguousarray(bias[n0:n1]).astype(np.float16)
        planes, packs = _prep_weights(qweight[:, n0:n1])
        in_maps.append({
            "wp": planes,
            "pk": packs,
            "cst": _prep_const(x, qz_slice, s_slice, b_slice),
        })
    return in_maps


def assemble_output(results, nloc=N_LOC, n_cores=N_CORES):
    parts = [np.asarray(results[core]["out"]) for core in range(n_cores)]
    return np.ascontiguousarray(np.concatenate(parts, axis=1))


def kernel(x, qweight, qzeros, scales, bias):
    from concourse.bass_utils import run_bass_kernel_spmd

    nc = _get_program()
    in_maps = make_in_maps(x, qweight, qzeros, scales, bias)
    res = run_bass_kernel_spmd(nc, in_maps, list(range(N_CORES)))
    return assemble_output(res.results)

